# revision 24
# baseline (speedup 1.0000x reference)
"""DISCO S2 convolution (nn_DISCOBlock_57801669869705) on 8 Trainium2 NeuronCores.

out[b,o,to,q] = sum_{c,k} w[o,c,k] * sum_{w,p} psi[k,to,w,p] * x[b,c,ti[to,w],(p+q)%P]

Mapping: for each output latitude row `to` and each active longitude-shift tap
(latitude-pair j, dp), a TensorE matmul accumulates into PSUM:
    out[:, (q,b)] += WPsi[(m,c), o].T @ xg[(m,c), (q+dp, b)]
with contraction over 128 partitions = (pair member m, channel c).
WPsi[(m,c), o] = sum_k psi[k,to,w(j,m),dp] * weight[o,c,k] is a host-side
transform of the small weight tensor; xg holds the latitudinally gathered,
longitudinally haloed input rows (host-side layout of x), both in bf16.

Tap pairing: adjacent taps (j,dp) and (j,dp+1) share xg rows, so they are
fused into ONE matmul with M=128 = (o, which-tap): PSUM rows 0:64 hold tap
dp's output, rows 64:128 hold tap dp+1's output over an N=362 window; the
host merges the halves with a 2-column shifted add.

Core-divergent schedules: instead of a shared SPMD tap template (union over
the 8 rows that would share a slot, ~1495 taps/core), the program is an
8-way tc.Switch on partition_id; each arm runs exactly its own rows' taps
(LPT-balanced, <= ~960 taps/core). The gathered-input (xg) layout and DMAs
are uniform across cores and stay outside the switch; the per-arm bodies
hold the matmul stream, the weight-chunk DMAs (alternating two HWDGE
rings), the PSUM->SBUF copies, and the output DMAs.
"""

import math
from functools import lru_cache

import numpy as np

B, C, O = 2, 64, 64
NLAT, P = 91, 180
NR, NPHI = 5, 6
K = (NR - 1) * NPHI + 1
NCORE = 8
NPHASE = 12  # max rows per core (8*12 >= 91)
NJ = 5  # pair groups per latitude window (4 pairs + 1 single)
NPAIR = 362  # moving-dim width of a paired matmul: B*(P+1)
WP_CHUNK = 8192  # wp cols per streamed weight-block DMA (16KB/partition bf16)
WP_BUFS = 7  # outstanding chunk buffers (lets two DMA rings run ahead)
WP_HOIST = 4  # leading chunks DMA'd outside the switch (uniform prefix)


def _compute_psi():
    theta_cut = 4.0 * math.pi / (NLAT - 1)
    half = int(math.ceil(theta_cut / (math.pi / (NLAT - 1))))
    theta = np.pi * np.arange(NLAT) / (NLAT - 1)
    phi_in = 2.0 * np.pi * np.arange(P) / P
    offs = np.arange(-half, half + 1)
    ti_raw = np.arange(NLAT)[:, None] + offs[None, :]
    valid = (ti_raw >= 0) & (ti_raw < NLAT)
    ti_idx = np.clip(ti_raw, 0, NLAT - 1)
    to = theta[:, None, None]
    ti = theta[ti_idx][:, :, None]
    ph = phi_in[None, None, :]
    xx = np.cos(to) * np.sin(ti) * np.cos(ph) - np.sin(to) * np.cos(ti)
    yy = np.sin(ti) * np.sin(ph)
    zz = np.sin(to) * np.sin(ti) * np.cos(ph) + np.cos(to) * np.cos(ti)
    r = np.arccos(np.clip(zz, -1.0, 1.0))
    az = np.mod(np.arctan2(yy, xx), 2.0 * np.pi)
    dr = theta_cut / (NR - 1)
    dphi = 2.0 * np.pi / NPHI
    inside = (r <= theta_cut) & valid[:, :, None]
    psi = np.zeros((K,) + r.shape)
    psi[0] = np.where(inside, np.maximum(0.0, 1.0 - r / dr), 0.0)
    for ir in range(1, NR):
        rad = np.maximum(0.0, 1.0 - np.abs(r - ir * dr) / dr)
        for ip in range(NPHI):
            d = np.abs(np.mod(az - ip * dphi + np.pi, 2.0 * np.pi) - np.pi)
            ang = np.maximum(0.0, 1.0 - d / dphi)
            psi[1 + (ir - 1) * NPHI + ip] = np.where(inside, rad * ang, 0.0)
    quad = np.sin(theta) * (np.pi / (NLAT - 1)) * (2.0 * np.pi / P)
    psi = psi * quad[ti_idx][None, :, :, None]
    return psi.astype(np.float32), ti_idx.astype(np.int32), 2 * half + 1


def _best_matching(u):
    """u: [W, P] bool. Return (cost, groups) — 4 pairs + 1 single over w=0..8
    minimizing sum over groups of |union of member activity|."""
    Wn = u.shape[0]
    M = np.zeros((Wn, Wn), dtype=np.int64)
    for a in range(Wn):
        for b in range(a + 1, Wn):
            M[a, b] = int((u[a] | u[b]).sum())
    s = np.array([int(u[w].sum()) for w in range(Wn)])
    INF = 10**12

    @lru_cache(maxsize=None)
    def f(mask, single_used):
        if mask == 0:
            return 0, ()
        a = (mask & -mask).bit_length() - 1
        rest = mask & ~(1 << a)
        best = (INF, ())
        for b in range(a + 1, Wn):
            if rest >> b & 1:
                c, pl = f(rest & ~(1 << b), single_used)
                if M[a, b] + c < best[0]:
                    best = (M[a, b] + c, pl + ((a, b),))
        if not single_used:
            c, pl = f(rest, True)
            if s[a] + c < best[0]:
                best = (s[a] + c, pl + ((a, None),))
        return best

    c, pl = f((1 << Wn) - 1, False)
    f.cache_clear()
    return c, list(pl)


def _build_plan():
    psi, ti_idx, W = _compute_psi()
    dpval = np.where(np.arange(P) < P // 2, np.arange(P), np.arange(P) - P)
    active = (psi != 0).any(axis=0)  # [To, W, P]

    # exact per-row pairing of window rows and tap lists
    row_groups, row_taps, row_cyc = {}, {}, {}
    for r in range(NLAT):
        _, groups = _best_matching(active[r])
        taps = []  # (j, dp)
        cyc = 0
        for j, (wa, wb) in enumerate(groups):
            ws = [w for w in (wa, wb) if w is not None]
            act = active[r][ws].any(axis=0)
            dps = sorted(dpval[np.nonzero(act)[0]].tolist())
            for dp_ in dps:
                taps.append((j, dp_))
            i = 0
            while i < len(dps):
                if i + 1 < len(dps) and dps[i + 1] == dps[i] + 1:
                    cyc += NPAIR
                    i += 2
                else:
                    cyc += B * P
                    i += 1
        row_groups[r] = groups
        row_taps[r] = taps
        row_cyc[r] = cyc

    # LPT assignment of rows to cores (minimize max core cycles, <= NPHASE)
    order = sorted(range(NLAT), key=lambda r: -row_cyc[r])
    loads = [0] * NCORE
    rows_of = [[] for _ in range(NCORE)]
    for r in order:
        cands = [c for c in range(NCORE) if len(rows_of[c]) < NPHASE]
        c = min(cands, key=lambda c_: loads[c_])
        loads[c] += row_cyc[r]
        rows_of[c].append(r)
    # heavy phases first within each core
    for c in range(NCORE):
        rows_of[c].sort(key=lambda r: -row_cyc[r])

    # uniform per-phase xg geometry: QP[s] = max over cores of that phase
    # row's padded circle width
    halos = {r: max((abs(d) for _, d in row_taps[r]), default=0)
             for r in range(NLAT)}
    QP = []
    for s in range(NPHASE):
        qp = max((P + 2 * halos[rows_of[c][s]]
                  for c in range(NCORE) if s < len(rows_of[c])), default=P)
        QP.append(qp)
    xoff = np.cumsum([0] + [NJ * B * qp for qp in QP]).tolist()
    XG_COLS = int(xoff[-1])

    # uniform chunk boundaries (graduated prefix); per-core streams pad
    # blocks up to the next boundary so no matmul block straddles one
    ubounds = [0, 512, 1536, 3584, 7680]
    while ubounds[-1] < 70000:
        ubounds.append(ubounds[-1] + WP_CHUNK)

    core_descs, core_wp_cols = [], []
    for c in range(NCORE):
        descs = []  # per phase: list of (kind, j, dp, wp_col)
        wp_col = 0
        ub_i = 1

        def _place(ncols):
            nonlocal wp_col, ub_i
            if wp_col < ubounds[ub_i] < wp_col + ncols:
                wp_col = ubounds[ub_i]
            while wp_col >= ubounds[ub_i]:
                ub_i += 1
            col = wp_col
            wp_col += ncols
            return col

        for s, r in enumerate(rows_of[c]):
            from collections import defaultdict
            byj = defaultdict(list)
            for j, dp_ in row_taps[r]:
                byj[j].append(dp_)
            prs, sgl = [], []
            for j in sorted(byj):
                dps = sorted(byj[j])
                i = 0
                while i < len(dps):
                    if i + 1 < len(dps) and dps[i + 1] == dps[i] + 1:
                        prs.append((j, dps[i]))
                        i += 2
                    else:
                        sgl.append((j, dps[i]))
                        i += 1
            assert prs, f"core {c} phase {s} row {r} has no paired tap"
            ph = []
            for j, dp_ in prs:
                ph.append(("P", j, dp_, _place(2 * O)))
            for j, dp_ in sgl:
                ph.append(("S", j, dp_, _place(O)))
            descs.append(ph)
        core_descs.append(descs)
        core_wp_cols.append(wp_col)

    WP_COLS = max(core_wp_cols)
    return dict(psi=psi, ti_idx=ti_idx, W=W, rows_of=rows_of,
                row_groups=row_groups, row_taps=row_taps, halos=halos,
                QP=QP, xoff=xoff, xg_cols=XG_COLS, core_descs=core_descs,
                core_wp_cols=core_wp_cols, wp_cols=int(WP_COLS),
                ubounds=ubounds)


_PLAN = None
_NC = None


def _get_plan():
    global _PLAN
    if _PLAN is None:
        _PLAN = _build_plan()
    return _PLAN


def _build_nc(plan):
    import concourse.bacc as bacc
    import concourse.mybir as mybir
    import concourse.tile as tile

    f32 = mybir.dt.float32
    bf16 = mybir.dt.bfloat16

    rows_of = plan["rows_of"]
    halos = plan["halos"]
    QP = plan["QP"]
    xoff = plan["xoff"]
    XG_COLS = plan["xg_cols"]
    WP_COLS = plan["wp_cols"]
    core_descs = plan["core_descs"]
    ubounds = plan["ubounds"]

    nc = bacc.Bacc("TRN2", target_bir_lowering=False, debug=False,
                   num_devices=NCORE)
    xg_d = nc.declare_dram_parameter("xg", [128, XG_COLS], bf16, isOutput=False)
    wp_d = nc.declare_dram_parameter("wp", [128, WP_COLS], bf16, isOutput=False)
    out_d = nc.declare_dram_parameter("out", [128, NPHASE * NPAIR], f32,
                                      isOutput=True)

    with tile.TileContext(nc) as tc:
        with (
            tc.tile_pool(name="xg", bufs=1) as xgp,
            tc.tile_pool(name="wp", bufs=WP_BUFS) as wpp,
            tc.tile_pool(name="ps", bufs=4, space="PSUM") as psp,
            tc.tile_pool(name="outp", bufs=1) as outp,
        ):
            # phase-0 xg pieces ride the fast HWDGE rings ahead of the
            # weight chunks so the very first matmul starts ASAP; later
            # phases stream on the gpsimd (SWDGE) queue in the background.
            xg_ts = []
            qp0 = QP[0]
            pieces = []
            for j in range(NJ):
                pj = xgp.tile([128, B * qp0], bf16, tag=f"xg0_{j}")
                eng = nc.sync if j == 0 else nc.scalar
                eng.dma_start(
                    pj[:], xg_d[:, xoff[0] + j * B * qp0:
                                xoff[0] + (j + 1) * B * qp0])
                pieces.append(pj)
            xg_ts.append(pieces)

            # leading wp chunks have uniform boundaries across cores, so
            # their DMAs can issue before the switch dispatch
            wp_pre = []
            for k in range(WP_HOIST):
                t = wpp.tile([128, WP_CHUNK], bf16, tag="wp")
                eng = nc.sync if k % 2 == 0 else nc.scalar
                eng.dma_start(t[:, :ubounds[k + 1] - ubounds[k]],
                              wp_d[:, ubounds[k]:ubounds[k + 1]])
                wp_pre.append(t)

            for s in range(1, NPHASE):
                qp = QP[s]
                seg = xgp.tile([128, NJ * B * qp], bf16, tag=f"xg{s}")
                nc.gpsimd.dma_start(seg[:], xg_d[:, xoff[s]:xoff[s + 1]])
                xg_ts.append(seg)
            out_t = outp.tile([128, NPHASE * NPAIR], f32)

            ET = mybir.EngineType
            index = {
                ET.PE: nc.tensor.partition_id(),
                ET.SP: nc.sync.partition_id(),
                ET.Activation: nc.scalar.partition_id(),
                ET.DVE: nc.vector.partition_id(),
                ET.Pool: nc.gpsimd.partition_id(),
            }
            for core in tc.Switch(index, NCORE):
                descs = core_descs[core]
                wp_end = plan["core_wp_cols"][core]
                cidx = 0
                wp_t = None
                for s in range(len(descs)):
                    r = rows_of[core][s]
                    h = halos[r]
                    qp = QP[s]
                    acc = psp.tile([128, NPAIR], f32)
                    ph = descs[s]
                    for i, (kind, j, dp, col) in enumerate(ph):
                        ncols = 2 * O if kind == "P" else O
                        while col >= ubounds[cidx]:
                            if cidx < WP_HOIST:
                                wp_t = wp_pre[cidx]
                            else:
                                ccols = (min(ubounds[cidx + 1], wp_end)
                                         - ubounds[cidx])
                                wp_t = wpp.tile([128, WP_CHUNK], bf16,
                                                tag="wp")
                                eng = nc.sync if cidx % 2 == 0 else nc.scalar
                                eng.dma_start(
                                    wp_t[:, :ccols],
                                    wp_d[:, ubounds[cidx]:
                                         ubounds[cidx] + ccols])
                            cidx += 1
                        coff = col - ubounds[cidx - 1]
                        lhsT = wp_t[:, coff:coff + ncols]
                        if s == 0:
                            xv = xg_ts[0][j]
                            base = B * (h + dp)
                        else:
                            xv = xg_ts[s]
                            base = j * B * qp + B * (h + dp)
                        if kind == "P":
                            rhs = xv[:, base: base + NPAIR]
                            out_ap = acc[:, :]
                        else:
                            rhs = xv[:, base: base + B * P]
                            out_ap = acc[0:O, 0:B * P]
                        nc.tensor.matmul(out_ap, lhsT, rhs,
                                         start=(i == 0), stop=(i == len(ph) - 1))
                    nc.vector.tensor_copy(
                        out_t[:, s * NPAIR:(s + 1) * NPAIR], acc[:, :])
                    oeng = nc.scalar if s % 2 == 0 else nc.sync
                    oeng.dma_start(
                        out_d[:, s * NPAIR:(s + 1) * NPAIR],
                        out_t[:, s * NPAIR:(s + 1) * NPAIR])

    nc.compile()
    return nc


def _get_nc():
    global _NC
    if _NC is None:
        _NC = _build_nc(_get_plan())
    return _NC


def _build_core_inputs(plan, x, weight):
    import ml_dtypes

    psi = plan["psi"]
    ti_idx = plan["ti_idx"]
    rows_of = plan["rows_of"]
    row_groups = plan["row_groups"]
    halos = plan["halos"]
    QP = plan["QP"]
    xoff = plan["xoff"]
    XG_COLS = plan["xg_cols"]
    WP_COLS = plan["wp_cols"]
    core_descs = plan["core_descs"]

    wk = np.ascontiguousarray(weight.transpose(2, 1, 0)).reshape(K, C, O)

    xgs, wps = [], []
    for core in range(NCORE):
        descs = core_descs[core]
        # tap coefficient table + target column per 64-col block
        blk_cols = []
        coef_list = []  # [2, K] per 64-col block
        for s, ph in enumerate(descs):
            r = rows_of[core][s]
            groups = row_groups[r]
            for kind, j, dp, col in ph:
                dps = (dp, dp + 1) if kind == "P" else (dp,)
                for bi, dp_ in enumerate(dps):
                    p = dp_ % P
                    cf = np.zeros((2, K), dtype=np.float32)
                    for m in range(2):
                        w_ = groups[j][m] if m < len(groups[j]) else None
                        if w_ is not None:
                            cf[m] = psi[:, r, w_, p]
                    coef_list.append(cf)
                    blk_cols.append(col + bi * O)
        coef = np.stack(coef_list)  # [nblk, 2, K]
        blocks = np.einsum("tmk,kco->tmco", coef, wk,
                           optimize=True).reshape(-1, 128, O)
        wp_full = np.zeros((128, WP_COLS), dtype=np.float32)
        for bi, col in enumerate(blk_cols):
            wp_full[:, col:col + O] = blocks[bi]
        wps.append(wp_full.astype(ml_dtypes.bfloat16))

        xg = np.zeros((128, XG_COLS), dtype=np.float32)
        for s, ph in enumerate(descs):
            r = rows_of[core][s]
            groups = row_groups[r]
            h = halos[r]
            qp_r = P + 2 * h
            qq = (np.arange(qp_r) - h) % P
            for j, members in enumerate(groups):
                for m in range(2):
                    w_ = members[m] if m < len(members) else None
                    if w_ is None:
                        continue
                    ti = ti_idx[r, w_]
                    blk = x[:, :, ti, :][:, :, qq]  # [b, c, qp_r]
                    base = xoff[s] + j * B * QP[s]
                    xg[m * 64:(m + 1) * 64, base: base + B * qp_r] = (
                        blk.transpose(1, 2, 0).reshape(C, qp_r * B))
        xgs.append(xg.astype(ml_dtypes.bfloat16))
    return xgs, wps


def kernel(x, weight):
    from concourse.bass_utils import run_bass_kernel_spmd

    x = np.ascontiguousarray(np.asarray(x, dtype=np.float32))
    weight = np.ascontiguousarray(np.asarray(weight, dtype=np.float32))
    plan = _get_plan()
    nc = _get_nc()
    xgs, wps = _build_core_inputs(plan, x, weight)
    in_maps = [{"xg": xgs[i], "wp": wps[i]} for i in range(NCORE)]
    res = run_bass_kernel_spmd(nc, in_maps, list(range(NCORE)))

    out = np.zeros((B, O, NLAT, P), dtype=np.float32)
    rows_of = plan["rows_of"]
    for core in range(NCORE):
        oc = np.asarray(res.results[core]["out"]).reshape(128, NPHASE, NPAIR)
        merged = oc[0:O, :, 0:B * P] + oc[O:128, :, B:B * P + B]
        mg = merged.reshape(O, NPHASE, P, B)
        for s, r in enumerate(rows_of[core]):
            out[:, :, r, :] = mg[:, s, :, :].transpose(2, 0, 1)
    return out


def _numpy_sim(x, weight):
    """Host replica of the device program (for validation)."""
    plan = _get_plan()
    xgs, wps = _build_core_inputs(plan, x, weight)
    rows_of = plan["rows_of"]
    halos = plan["halos"]
    QP = plan["QP"]
    xoff = plan["xoff"]
    core_descs = plan["core_descs"]
    out = np.zeros((B, O, NLAT, P), dtype=np.float32)
    for core in range(NCORE):
        xg = xgs[core].astype(np.float32)
        wp = wps[core].astype(np.float32)
        descs = core_descs[core]
        for s, ph in enumerate(descs):
            r = rows_of[core][s]
            h = halos[r]
            qp = QP[s]
            acc = np.zeros((128, NPAIR), dtype=np.float32)
            for (kind, j, dp, col) in ph:
                base = xoff[s] + j * B * qp + B * (h + dp)
                if kind == "P":
                    acc += wp[:, col:col + 2 * O].T @ xg[:, base: base + NPAIR]
                else:
                    acc[0:O, 0:B * P] += (
                        wp[:, col:col + O].T @ xg[:, base: base + B * P])
            merged = acc[0:O, 0:B * P] + acc[O:128, B:B * P + B]
            out[:, :, r, :] = merged.reshape(O, P, B).transpose(2, 0, 1)
    return out


if __name__ == "__main__":
    plan = _get_plan()
    for c in range(NCORE):
        ph = plan["core_descs"][c]
        npair_ = sum(1 for p_ in ph for d in p_ if d[0] == "P")
        nsgl = sum(1 for p_ in ph for d in p_ if d[0] == "S")
        cyc = npair_ * NPAIR + nsgl * B * P
        print(f"core {c}: rows={len(ph)} pairs={npair_} singles={nsgl} "
              f"cyc={cyc} -> {cyc*0.41667/1e3:.1f}us wp_MB="
              f"{plan['core_wp_cols'][c]*128*2/1e6:.1f}")
    print("xg MB:", plan["xg_cols"] * 128 * 2 / 1e6,
          "wp decl MB:", plan["wp_cols"] * 128 * 2 / 1e6)
    d = np.load("/tmp/ref_io.npz")
    got = _numpy_sim(d["x"], d["weight"])
    exp = d["expected"]
    rel = np.linalg.norm((got - exp).ravel()) / np.linalg.norm(exp.ravel())
    print("numpy-sim rel err:", rel)


# revision 26
# speedup vs baseline: 1.1052x; 1.1052x over previous
"""DISCO S2 convolution (nn_DISCOBlock_57801669869705) on 8 Trainium2 NeuronCores.

out[b,o,to,q] = sum_{c,k} w[o,c,k] * sum_{w,p} psi[k,to,w,p] * x[b,c,ti[to,w],(p+q)%P]

Mapping: for each output latitude row `to` and each active longitude-shift tap
(latitude-pair j, dp), a TensorE matmul accumulates into PSUM:
    out[:, (q,b)] += WPsi[(m,c), o].T @ xg[(m,c), (q+dp, b)]
with contraction over 128 partitions = (pair member m, channel c).
WPsi[(m,c), o] = sum_k psi[k,to,w(j,m),dp] * weight[o,c,k] is a host-side
transform of the small weight tensor; xg holds the latitudinally gathered,
longitudinally haloed input rows (host-side layout of x), both in bf16.

Tap pairing: adjacent taps (j,dp) and (j,dp+1) share xg rows, so they are
fused into ONE matmul with M=128 = (o, which-tap): PSUM rows 0:64 hold tap
dp's output, rows 64:128 hold tap dp+1's output over an N=362 window; the
host merges the halves with a 2-column shifted add.

Core-divergent schedules: instead of a shared SPMD tap template (union over
the 8 rows that would share a slot, ~1495 taps/core), the program is an
8-way tc.Switch on partition_id; each arm runs exactly its own rows' taps
(LPT-balanced, <= ~960 taps/core). The gathered-input (xg) layout and DMAs
are uniform across cores and stay outside the switch; the per-arm bodies
hold the matmul stream, the weight-chunk DMAs (alternating two HWDGE
rings), the PSUM->SBUF copies, and the output DMAs.
"""

import math
from functools import lru_cache

import numpy as np

B, C, O = 2, 64, 64
NLAT, P = 91, 180
NR, NPHI = 5, 6
K = (NR - 1) * NPHI + 1
NCORE = 8
NPHASE = 12  # max rows per core (8*12 >= 91)
NJ = 5  # pair groups per latitude window (4 pairs + 1 single)
NPAIR = 362  # moving-dim width of a paired matmul: B*(P+1)
WP_CHUNK = 8192  # wp cols per streamed weight-block DMA (16KB/partition bf16)
WP_BUFS = 7  # outstanding chunk buffers (lets two DMA rings run ahead)
WP_HOIST = 4  # leading chunks DMA'd outside the switch (uniform prefix)


def _compute_psi():
    theta_cut = 4.0 * math.pi / (NLAT - 1)
    half = int(math.ceil(theta_cut / (math.pi / (NLAT - 1))))
    theta = np.pi * np.arange(NLAT) / (NLAT - 1)
    phi_in = 2.0 * np.pi * np.arange(P) / P
    offs = np.arange(-half, half + 1)
    ti_raw = np.arange(NLAT)[:, None] + offs[None, :]
    valid = (ti_raw >= 0) & (ti_raw < NLAT)
    ti_idx = np.clip(ti_raw, 0, NLAT - 1)
    to = theta[:, None, None]
    ti = theta[ti_idx][:, :, None]
    ph = phi_in[None, None, :]
    xx = np.cos(to) * np.sin(ti) * np.cos(ph) - np.sin(to) * np.cos(ti)
    yy = np.sin(ti) * np.sin(ph)
    zz = np.sin(to) * np.sin(ti) * np.cos(ph) + np.cos(to) * np.cos(ti)
    r = np.arccos(np.clip(zz, -1.0, 1.0))
    az = np.mod(np.arctan2(yy, xx), 2.0 * np.pi)
    dr = theta_cut / (NR - 1)
    dphi = 2.0 * np.pi / NPHI
    inside = (r <= theta_cut) & valid[:, :, None]
    psi = np.zeros((K,) + r.shape)
    psi[0] = np.where(inside, np.maximum(0.0, 1.0 - r / dr), 0.0)
    for ir in range(1, NR):
        rad = np.maximum(0.0, 1.0 - np.abs(r - ir * dr) / dr)
        for ip in range(NPHI):
            d = np.abs(np.mod(az - ip * dphi + np.pi, 2.0 * np.pi) - np.pi)
            ang = np.maximum(0.0, 1.0 - d / dphi)
            psi[1 + (ir - 1) * NPHI + ip] = np.where(inside, rad * ang, 0.0)
    quad = np.sin(theta) * (np.pi / (NLAT - 1)) * (2.0 * np.pi / P)
    psi = psi * quad[ti_idx][None, :, :, None]
    return psi.astype(np.float32), ti_idx.astype(np.int32), 2 * half + 1


def _best_matching(u):
    """u: [W, P] bool. Return (cost, groups) — 4 pairs + 1 single over w=0..8
    minimizing sum over groups of |union of member activity|."""
    Wn = u.shape[0]
    M = np.zeros((Wn, Wn), dtype=np.int64)
    for a in range(Wn):
        for b in range(a + 1, Wn):
            M[a, b] = int((u[a] | u[b]).sum())
    s = np.array([int(u[w].sum()) for w in range(Wn)])
    INF = 10**12

    @lru_cache(maxsize=None)
    def f(mask, single_used):
        if mask == 0:
            return 0, ()
        a = (mask & -mask).bit_length() - 1
        rest = mask & ~(1 << a)
        best = (INF, ())
        for b in range(a + 1, Wn):
            if rest >> b & 1:
                c, pl = f(rest & ~(1 << b), single_used)
                if M[a, b] + c < best[0]:
                    best = (M[a, b] + c, pl + ((a, b),))
        if not single_used:
            c, pl = f(rest, True)
            if s[a] + c < best[0]:
                best = (s[a] + c, pl + ((a, None),))
        return best

    c, pl = f((1 << Wn) - 1, False)
    f.cache_clear()
    return c, list(pl)


def _build_plan():
    psi, ti_idx, W = _compute_psi()
    dpval = np.where(np.arange(P) < P // 2, np.arange(P), np.arange(P) - P)
    active = (psi != 0).any(axis=0)  # [To, W, P]

    # exact per-row pairing of window rows and tap lists
    row_groups, row_taps, row_cyc = {}, {}, {}
    for r in range(NLAT):
        _, groups = _best_matching(active[r])
        taps = []  # (j, dp)
        cyc = 0
        for j, (wa, wb) in enumerate(groups):
            ws = [w for w in (wa, wb) if w is not None]
            act = active[r][ws].any(axis=0)
            dps = sorted(dpval[np.nonzero(act)[0]].tolist())
            for dp_ in dps:
                taps.append((j, dp_))
            i = 0
            while i < len(dps):
                if i + 1 < len(dps) and dps[i + 1] == dps[i] + 1:
                    cyc += NPAIR
                    i += 2
                else:
                    cyc += B * P
                    i += 1
        row_groups[r] = groups
        row_taps[r] = taps
        row_cyc[r] = cyc

    # LPT assignment of rows to cores (minimize max core cycles, <= NPHASE)
    order = sorted(range(NLAT), key=lambda r: -row_cyc[r])
    loads = [0] * NCORE
    rows_of = [[] for _ in range(NCORE)]
    for r in order:
        cands = [c for c in range(NCORE) if len(rows_of[c]) < NPHASE]
        c = min(cands, key=lambda c_: loads[c_])
        loads[c] += row_cyc[r]
        rows_of[c].append(r)
    # heavy phases first within each core
    for c in range(NCORE):
        rows_of[c].sort(key=lambda r: -row_cyc[r])

    # uniform per-phase xg geometry: QP[s] = max over cores of that phase
    # row's padded circle width
    halos = {r: max((abs(d) for _, d in row_taps[r]), default=0)
             for r in range(NLAT)}
    QP = []
    for s in range(NPHASE):
        qp = max((P + 2 * halos[rows_of[c][s]]
                  for c in range(NCORE) if s < len(rows_of[c])), default=P)
        QP.append(qp)
    xoff = np.cumsum([0] + [NJ * B * qp for qp in QP]).tolist()
    XG_COLS = int(xoff[-1])

    # uniform chunk boundaries (graduated prefix); per-core streams pad
    # blocks up to the next boundary so no matmul block straddles one
    ubounds = [0, 512, 1536, 3584, 7680]
    while ubounds[-1] < 70000:
        ubounds.append(ubounds[-1] + WP_CHUNK)

    core_descs, core_wp_cols = [], []
    for c in range(NCORE):
        descs = []  # per phase: list of (kind, j, dp, wp_col)
        wp_col = 0
        ub_i = 1

        def _place(ncols):
            nonlocal wp_col, ub_i
            if wp_col < ubounds[ub_i] < wp_col + ncols:
                wp_col = ubounds[ub_i]
            while wp_col >= ubounds[ub_i]:
                ub_i += 1
            col = wp_col
            wp_col += ncols
            return col

        for s, r in enumerate(rows_of[c]):
            from collections import defaultdict
            byj = defaultdict(list)
            for j, dp_ in row_taps[r]:
                byj[j].append(dp_)
            prs, sgl = [], []
            for j in sorted(byj):
                dps = sorted(byj[j])
                i = 0
                while i < len(dps):
                    if i + 1 < len(dps) and dps[i + 1] == dps[i] + 1:
                        prs.append((j, dps[i]))
                        i += 2
                    else:
                        sgl.append((j, dps[i]))
                        i += 1
            assert prs, f"core {c} phase {s} row {r} has no paired tap"
            ph = []
            for j, dp_ in prs:
                ph.append(("P", j, dp_, _place(2 * O)))
            for j, dp_ in sgl:
                ph.append(("S", j, dp_, _place(O)))
            descs.append(ph)
        core_descs.append(descs)
        core_wp_cols.append(wp_col)

    WP_COLS = max(core_wp_cols)
    return dict(psi=psi, ti_idx=ti_idx, W=W, rows_of=rows_of,
                row_groups=row_groups, row_taps=row_taps, halos=halos,
                QP=QP, xoff=xoff, xg_cols=XG_COLS, core_descs=core_descs,
                core_wp_cols=core_wp_cols, wp_cols=int(WP_COLS),
                ubounds=ubounds)


_PLAN = None
_NC = None


def _get_plan():
    global _PLAN
    if _PLAN is None:
        _PLAN = _build_plan()
    return _PLAN


def _build_nc(plan):
    import concourse.bacc as bacc
    import concourse.mybir as mybir
    import concourse.tile as tile

    f32 = mybir.dt.float32
    bf16 = mybir.dt.bfloat16

    rows_of = plan["rows_of"]
    halos = plan["halos"]
    QP = plan["QP"]
    xoff = plan["xoff"]
    XG_COLS = plan["xg_cols"]
    WP_COLS = plan["wp_cols"]
    core_descs = plan["core_descs"]
    ubounds = plan["ubounds"]

    nc = bacc.Bacc("TRN2", target_bir_lowering=False, debug=False,
                   num_devices=NCORE)
    xg_d = nc.declare_dram_parameter("xg", [128, XG_COLS], bf16, isOutput=False)
    wp_d = nc.declare_dram_parameter("wp", [128, WP_COLS], bf16, isOutput=False)
    out_d = nc.declare_dram_parameter("out", [128, NPHASE * NPAIR], f32,
                                      isOutput=True)

    with tile.TileContext(nc) as tc:
        with (
            tc.tile_pool(name="xg", bufs=1) as xgp,
            tc.tile_pool(name="wp", bufs=WP_BUFS) as wpp,
            tc.tile_pool(name="ps", bufs=4, space="PSUM") as psp,
            tc.tile_pool(name="outp", bufs=1) as outp,
        ):
            # the first two phase-0 xg pieces ride the fast HWDGE rings
            # ahead of the weight chunks so the first matmuls start ASAP;
            # the rest streams on the gpsimd (SWDGE) queue in the
            # background.
            xg_ts = []
            qp0 = QP[0]
            pieces = []
            for j in range(NJ):
                pj = xgp.tile([128, B * qp0], bf16, tag=f"xg0_{j}")
                eng = (nc.sync, nc.scalar, nc.gpsimd, nc.gpsimd,
                       nc.gpsimd)[j]
                eng.dma_start(
                    pj[:], xg_d[:, xoff[0] + j * B * qp0:
                                xoff[0] + (j + 1) * B * qp0])
                pieces.append(pj)
            xg_ts.append(pieces)

            # leading wp chunks have uniform boundaries across cores, so
            # their DMAs can issue before the switch dispatch
            wp_pre = []
            for k in range(WP_HOIST):
                t = wpp.tile([128, WP_CHUNK], bf16, tag="wp")
                eng = nc.sync if k % 2 == 0 else nc.scalar
                eng.dma_start(t[:, :ubounds[k + 1] - ubounds[k]],
                              wp_d[:, ubounds[k]:ubounds[k + 1]])
                wp_pre.append(t)

            for s in range(1, NPHASE):
                qp = QP[s]
                seg = xgp.tile([128, NJ * B * qp], bf16, tag=f"xg{s}")
                nc.gpsimd.dma_start(seg[:], xg_d[:, xoff[s]:xoff[s + 1]])
                xg_ts.append(seg)
            out_t = outp.tile([128, NPHASE * NPAIR], f32)

            ET = mybir.EngineType
            index = {
                ET.PE: nc.tensor.partition_id(),
                ET.SP: nc.sync.partition_id(),
                ET.Activation: nc.scalar.partition_id(),
                ET.DVE: nc.vector.partition_id(),
                ET.Pool: nc.gpsimd.partition_id(),
            }
            for core in tc.Switch(index, NCORE):
                descs = core_descs[core]
                wp_end = plan["core_wp_cols"][core]
                cidx = 0
                wp_t = None
                for s in range(len(descs)):
                    r = rows_of[core][s]
                    h = halos[r]
                    qp = QP[s]
                    acc = psp.tile([128, NPAIR], f32)
                    ph = descs[s]
                    for i, (kind, j, dp, col) in enumerate(ph):
                        ncols = 2 * O if kind == "P" else O
                        while col >= ubounds[cidx]:
                            if cidx < WP_HOIST:
                                wp_t = wp_pre[cidx]
                            else:
                                ccols = (min(ubounds[cidx + 1], wp_end)
                                         - ubounds[cidx])
                                wp_t = wpp.tile([128, WP_CHUNK], bf16,
                                                tag="wp")
                                eng = nc.sync if cidx % 2 == 0 else nc.scalar
                                eng.dma_start(
                                    wp_t[:, :ccols],
                                    wp_d[:, ubounds[cidx]:
                                         ubounds[cidx] + ccols])
                            cidx += 1
                        coff = col - ubounds[cidx - 1]
                        lhsT = wp_t[:, coff:coff + ncols]
                        if s == 0:
                            xv = xg_ts[0][j]
                            base = B * (h + dp)
                        else:
                            xv = xg_ts[s]
                            base = j * B * qp + B * (h + dp)
                        if kind == "P":
                            rhs = xv[:, base: base + NPAIR]
                            out_ap = acc[:, :]
                        else:
                            rhs = xv[:, base: base + B * P]
                            out_ap = acc[0:O, 0:B * P]
                        nc.tensor.matmul(out_ap, lhsT, rhs,
                                         start=(i == 0), stop=(i == len(ph) - 1))
                    nc.vector.tensor_copy(
                        out_t[:, s * NPAIR:(s + 1) * NPAIR], acc[:, :])
                    # late-phase outputs take the (by then idle) HWDGE
                    # rings so the tail isn't gated on the slow SWDGE queue
                    if s >= len(descs) - 2:
                        oeng = nc.scalar if s % 2 == 0 else nc.sync
                    else:
                        oeng = nc.gpsimd
                    oeng.dma_start(
                        out_d[:, s * NPAIR:(s + 1) * NPAIR],
                        out_t[:, s * NPAIR:(s + 1) * NPAIR])

    nc.compile()
    return nc


def _get_nc():
    global _NC
    if _NC is None:
        _NC = _build_nc(_get_plan())
    return _NC


def _build_core_inputs(plan, x, weight):
    import ml_dtypes

    psi = plan["psi"]
    ti_idx = plan["ti_idx"]
    rows_of = plan["rows_of"]
    row_groups = plan["row_groups"]
    halos = plan["halos"]
    QP = plan["QP"]
    xoff = plan["xoff"]
    XG_COLS = plan["xg_cols"]
    WP_COLS = plan["wp_cols"]
    core_descs = plan["core_descs"]

    wk = np.ascontiguousarray(weight.transpose(2, 1, 0)).reshape(K, C, O)

    xgs, wps = [], []
    for core in range(NCORE):
        descs = core_descs[core]
        # tap coefficient table + target column per 64-col block
        blk_cols = []
        coef_list = []  # [2, K] per 64-col block
        for s, ph in enumerate(descs):
            r = rows_of[core][s]
            groups = row_groups[r]
            for kind, j, dp, col in ph:
                dps = (dp, dp + 1) if kind == "P" else (dp,)
                for bi, dp_ in enumerate(dps):
                    p = dp_ % P
                    cf = np.zeros((2, K), dtype=np.float32)
                    for m in range(2):
                        w_ = groups[j][m] if m < len(groups[j]) else None
                        if w_ is not None:
                            cf[m] = psi[:, r, w_, p]
                    coef_list.append(cf)
                    blk_cols.append(col + bi * O)
        coef = np.stack(coef_list)  # [nblk, 2, K]
        blocks = np.einsum("tmk,kco->tmco", coef, wk,
                           optimize=True).reshape(-1, 128, O)
        wp_full = np.zeros((128, WP_COLS), dtype=np.float32)
        for bi, col in enumerate(blk_cols):
            wp_full[:, col:col + O] = blocks[bi]
        wps.append(wp_full.astype(ml_dtypes.bfloat16))

        xg = np.zeros((128, XG_COLS), dtype=np.float32)
        for s, ph in enumerate(descs):
            r = rows_of[core][s]
            groups = row_groups[r]
            h = halos[r]
            qp_r = P + 2 * h
            qq = (np.arange(qp_r) - h) % P
            for j, members in enumerate(groups):
                for m in range(2):
                    w_ = members[m] if m < len(members) else None
                    if w_ is None:
                        continue
                    ti = ti_idx[r, w_]
                    blk = x[:, :, ti, :][:, :, qq]  # [b, c, qp_r]
                    base = xoff[s] + j * B * QP[s]
                    xg[m * 64:(m + 1) * 64, base: base + B * qp_r] = (
                        blk.transpose(1, 2, 0).reshape(C, qp_r * B))
        xgs.append(xg.astype(ml_dtypes.bfloat16))
    return xgs, wps


def kernel(x, weight):
    from concourse.bass_utils import run_bass_kernel_spmd

    x = np.ascontiguousarray(np.asarray(x, dtype=np.float32))
    weight = np.ascontiguousarray(np.asarray(weight, dtype=np.float32))
    plan = _get_plan()
    nc = _get_nc()
    xgs, wps = _build_core_inputs(plan, x, weight)
    in_maps = [{"xg": xgs[i], "wp": wps[i]} for i in range(NCORE)]
    res = run_bass_kernel_spmd(nc, in_maps, list(range(NCORE)))

    out = np.zeros((B, O, NLAT, P), dtype=np.float32)
    rows_of = plan["rows_of"]
    for core in range(NCORE):
        oc = np.asarray(res.results[core]["out"]).reshape(128, NPHASE, NPAIR)
        merged = oc[0:O, :, 0:B * P] + oc[O:128, :, B:B * P + B]
        mg = merged.reshape(O, NPHASE, P, B)
        for s, r in enumerate(rows_of[core]):
            out[:, :, r, :] = mg[:, s, :, :].transpose(2, 0, 1)
    return out


def _numpy_sim(x, weight):
    """Host replica of the device program (for validation)."""
    plan = _get_plan()
    xgs, wps = _build_core_inputs(plan, x, weight)
    rows_of = plan["rows_of"]
    halos = plan["halos"]
    QP = plan["QP"]
    xoff = plan["xoff"]
    core_descs = plan["core_descs"]
    out = np.zeros((B, O, NLAT, P), dtype=np.float32)
    for core in range(NCORE):
        xg = xgs[core].astype(np.float32)
        wp = wps[core].astype(np.float32)
        descs = core_descs[core]
        for s, ph in enumerate(descs):
            r = rows_of[core][s]
            h = halos[r]
            qp = QP[s]
            acc = np.zeros((128, NPAIR), dtype=np.float32)
            for (kind, j, dp, col) in ph:
                base = xoff[s] + j * B * qp + B * (h + dp)
                if kind == "P":
                    acc += wp[:, col:col + 2 * O].T @ xg[:, base: base + NPAIR]
                else:
                    acc[0:O, 0:B * P] += (
                        wp[:, col:col + O].T @ xg[:, base: base + B * P])
            merged = acc[0:O, 0:B * P] + acc[O:128, B:B * P + B]
            out[:, :, r, :] = merged.reshape(O, P, B).transpose(2, 0, 1)
    return out


if __name__ == "__main__":
    plan = _get_plan()
    for c in range(NCORE):
        ph = plan["core_descs"][c]
        npair_ = sum(1 for p_ in ph for d in p_ if d[0] == "P")
        nsgl = sum(1 for p_ in ph for d in p_ if d[0] == "S")
        cyc = npair_ * NPAIR + nsgl * B * P
        print(f"core {c}: rows={len(ph)} pairs={npair_} singles={nsgl} "
              f"cyc={cyc} -> {cyc*0.41667/1e3:.1f}us wp_MB="
              f"{plan['core_wp_cols'][c]*128*2/1e6:.1f}")
    print("xg MB:", plan["xg_cols"] * 128 * 2 / 1e6,
          "wp decl MB:", plan["wp_cols"] * 128 * 2 / 1e6)
    d = np.load("/tmp/ref_io.npz")
    got = _numpy_sim(d["x"], d["weight"])
    exp = d["expected"]
    rel = np.linalg.norm((got - exp).ravel()) / np.linalg.norm(exp.ravel())
    print("numpy-sim rel err:", rel)


# revision 28
# speedup vs baseline: 1.2055x; 1.0907x over previous
"""DISCO S2 convolution (nn_DISCOBlock_57801669869705) on 8 Trainium2 NeuronCores.

out[b,o,to,q] = sum_{c,k} w[o,c,k] * sum_{w,p} psi[k,to,w,p] * x[b,c,ti[to,w],(p+q)%P]

Mapping: for each output latitude row `to` and each active longitude-shift tap
(latitude-pair j, dp), a TensorE matmul accumulates into PSUM:
    out[:, (q,b)] += WPsi[(m,c), o].T @ xg[(m,c), (q+dp, b)]
with contraction over 128 partitions = (pair member m, channel c).
WPsi[(m,c), o] = sum_k psi[k,to,w(j,m),dp] * weight[o,c,k] is a host-side
transform of the small weight tensor; xg holds the latitudinally gathered,
longitudinally haloed input rows (host-side layout of x), both in bf16.

Tap pairing: adjacent taps (j,dp) and (j,dp+1) share xg rows, so they are
fused into ONE matmul with M=128 = (o, which-tap): PSUM rows 0:64 hold tap
dp's output, rows 64:128 hold tap dp+1's output over an N=362 window; the
host merges the halves with a 2-column shifted add.

Core-divergent schedules: instead of a shared SPMD tap template (union over
the 8 rows that would share a slot, ~1495 taps/core), the program is an
8-way tc.Switch on partition_id; each arm runs exactly its own rows' taps
(LPT-balanced, <= ~960 taps/core). The gathered-input (xg) layout and DMAs
are uniform across cores and stay outside the switch; the per-arm bodies
hold the matmul stream, the weight-chunk DMAs (alternating two HWDGE
rings), the PSUM->SBUF copies, and the output DMAs.
"""

import math
from functools import lru_cache

import numpy as np

B, C, O = 2, 64, 64
NLAT, P = 91, 180
NR, NPHI = 5, 6
K = (NR - 1) * NPHI + 1
NCORE = 8
NPHASE = 12  # max rows per core (8*12 >= 91)
NJ = 5  # pair groups per latitude window (4 pairs + 1 single)
NPAIR = 362  # moving-dim width of a paired matmul: B*(P+1)
WP_CHUNK = 4096  # wp cols per streamed weight-block DMA (8KB/partition bf16)
WP_BUFS = 12  # outstanding chunk buffers (lets two DMA rings run ahead)
WP_HOIST = 6  # leading chunks DMA'd outside the switch (uniform prefix)


def _compute_psi():
    theta_cut = 4.0 * math.pi / (NLAT - 1)
    half = int(math.ceil(theta_cut / (math.pi / (NLAT - 1))))
    theta = np.pi * np.arange(NLAT) / (NLAT - 1)
    phi_in = 2.0 * np.pi * np.arange(P) / P
    offs = np.arange(-half, half + 1)
    ti_raw = np.arange(NLAT)[:, None] + offs[None, :]
    valid = (ti_raw >= 0) & (ti_raw < NLAT)
    ti_idx = np.clip(ti_raw, 0, NLAT - 1)
    to = theta[:, None, None]
    ti = theta[ti_idx][:, :, None]
    ph = phi_in[None, None, :]
    xx = np.cos(to) * np.sin(ti) * np.cos(ph) - np.sin(to) * np.cos(ti)
    yy = np.sin(ti) * np.sin(ph)
    zz = np.sin(to) * np.sin(ti) * np.cos(ph) + np.cos(to) * np.cos(ti)
    r = np.arccos(np.clip(zz, -1.0, 1.0))
    az = np.mod(np.arctan2(yy, xx), 2.0 * np.pi)
    dr = theta_cut / (NR - 1)
    dphi = 2.0 * np.pi / NPHI
    inside = (r <= theta_cut) & valid[:, :, None]
    psi = np.zeros((K,) + r.shape)
    psi[0] = np.where(inside, np.maximum(0.0, 1.0 - r / dr), 0.0)
    for ir in range(1, NR):
        rad = np.maximum(0.0, 1.0 - np.abs(r - ir * dr) / dr)
        for ip in range(NPHI):
            d = np.abs(np.mod(az - ip * dphi + np.pi, 2.0 * np.pi) - np.pi)
            ang = np.maximum(0.0, 1.0 - d / dphi)
            psi[1 + (ir - 1) * NPHI + ip] = np.where(inside, rad * ang, 0.0)
    quad = np.sin(theta) * (np.pi / (NLAT - 1)) * (2.0 * np.pi / P)
    psi = psi * quad[ti_idx][None, :, :, None]
    return psi.astype(np.float32), ti_idx.astype(np.int32), 2 * half + 1


def _best_matching(u):
    """u: [W, P] bool. Return (cost, groups) — 4 pairs + 1 single over w=0..8
    minimizing sum over groups of |union of member activity|."""
    Wn = u.shape[0]
    M = np.zeros((Wn, Wn), dtype=np.int64)
    for a in range(Wn):
        for b in range(a + 1, Wn):
            M[a, b] = int((u[a] | u[b]).sum())
    s = np.array([int(u[w].sum()) for w in range(Wn)])
    INF = 10**12

    @lru_cache(maxsize=None)
    def f(mask, single_used):
        if mask == 0:
            return 0, ()
        a = (mask & -mask).bit_length() - 1
        rest = mask & ~(1 << a)
        best = (INF, ())
        for b in range(a + 1, Wn):
            if rest >> b & 1:
                c, pl = f(rest & ~(1 << b), single_used)
                if M[a, b] + c < best[0]:
                    best = (M[a, b] + c, pl + ((a, b),))
        if not single_used:
            c, pl = f(rest, True)
            if s[a] + c < best[0]:
                best = (s[a] + c, pl + ((a, None),))
        return best

    c, pl = f((1 << Wn) - 1, False)
    f.cache_clear()
    return c, list(pl)


def _build_plan():
    psi, ti_idx, W = _compute_psi()
    dpval = np.where(np.arange(P) < P // 2, np.arange(P), np.arange(P) - P)
    active = (psi != 0).any(axis=0)  # [To, W, P]

    # exact per-row pairing of window rows and tap lists
    row_groups, row_taps, row_cyc = {}, {}, {}
    for r in range(NLAT):
        _, groups = _best_matching(active[r])
        taps = []  # (j, dp)
        cyc = 0
        for j, (wa, wb) in enumerate(groups):
            ws = [w for w in (wa, wb) if w is not None]
            act = active[r][ws].any(axis=0)
            dps = sorted(dpval[np.nonzero(act)[0]].tolist())
            for dp_ in dps:
                taps.append((j, dp_))
            i = 0
            while i < len(dps):
                if i + 1 < len(dps) and dps[i + 1] == dps[i] + 1:
                    cyc += NPAIR
                    i += 2
                else:
                    cyc += B * P
                    i += 1
        row_groups[r] = groups
        row_taps[r] = taps
        row_cyc[r] = cyc

    # LPT assignment of rows to cores (minimize max core cycles, <= NPHASE)
    order = sorted(range(NLAT), key=lambda r: -row_cyc[r])
    loads = [0] * NCORE
    rows_of = [[] for _ in range(NCORE)]
    for r in order:
        cands = [c for c in range(NCORE) if len(rows_of[c]) < NPHASE]
        c = min(cands, key=lambda c_: loads[c_])
        loads[c] += row_cyc[r]
        rows_of[c].append(r)
    # heavy phases first within each core
    for c in range(NCORE):
        rows_of[c].sort(key=lambda r: -row_cyc[r])

    # uniform per-phase xg geometry: QP[s] = max over cores of that phase
    # row's padded circle width
    halos = {r: max((abs(d) for _, d in row_taps[r]), default=0)
             for r in range(NLAT)}
    QP = []
    for s in range(NPHASE):
        qp = max((P + 2 * halos[rows_of[c][s]]
                  for c in range(NCORE) if s < len(rows_of[c])), default=P)
        QP.append(qp)
    xoff = np.cumsum([0] + [NJ * B * qp for qp in QP]).tolist()
    XG_COLS = int(xoff[-1])

    # uniform chunk boundaries (graduated prefix); per-core streams pad
    # blocks up to the next boundary so no matmul block straddles one
    ubounds = [0, 512, 1536, 3584]
    while ubounds[-1] < 70000:
        ubounds.append(ubounds[-1] + WP_CHUNK)

    core_descs, core_wp_cols = [], []
    for c in range(NCORE):
        descs = []  # per phase: list of (kind, j, dp, wp_col)
        wp_col = 0
        ub_i = 1

        def _place(ncols):
            nonlocal wp_col, ub_i
            if wp_col < ubounds[ub_i] < wp_col + ncols:
                wp_col = ubounds[ub_i]
            while wp_col >= ubounds[ub_i]:
                ub_i += 1
            col = wp_col
            wp_col += ncols
            return col

        for s, r in enumerate(rows_of[c]):
            from collections import defaultdict
            byj = defaultdict(list)
            for j, dp_ in row_taps[r]:
                byj[j].append(dp_)
            prs, sgl = [], []
            for j in sorted(byj):
                dps = sorted(byj[j])
                i = 0
                while i < len(dps):
                    if i + 1 < len(dps) and dps[i + 1] == dps[i] + 1:
                        prs.append((j, dps[i]))
                        i += 2
                    else:
                        sgl.append((j, dps[i]))
                        i += 1
            assert prs, f"core {c} phase {s} row {r} has no paired tap"
            ph = []
            for j, dp_ in prs:
                ph.append(("P", j, dp_, _place(2 * O)))
            for j, dp_ in sgl:
                ph.append(("S", j, dp_, _place(O)))
            descs.append(ph)
        core_descs.append(descs)
        core_wp_cols.append(wp_col)

    WP_COLS = max(core_wp_cols)
    return dict(psi=psi, ti_idx=ti_idx, W=W, rows_of=rows_of,
                row_groups=row_groups, row_taps=row_taps, halos=halos,
                QP=QP, xoff=xoff, xg_cols=XG_COLS, core_descs=core_descs,
                core_wp_cols=core_wp_cols, wp_cols=int(WP_COLS),
                ubounds=ubounds)


_PLAN = None
_NC = None


def _get_plan():
    global _PLAN
    if _PLAN is None:
        _PLAN = _build_plan()
    return _PLAN


def _build_nc(plan):
    import concourse.bacc as bacc
    import concourse.mybir as mybir
    import concourse.tile as tile

    f32 = mybir.dt.float32
    bf16 = mybir.dt.bfloat16

    rows_of = plan["rows_of"]
    halos = plan["halos"]
    QP = plan["QP"]
    xoff = plan["xoff"]
    XG_COLS = plan["xg_cols"]
    WP_COLS = plan["wp_cols"]
    core_descs = plan["core_descs"]
    ubounds = plan["ubounds"]

    nc = bacc.Bacc("TRN2", target_bir_lowering=False, debug=False,
                   num_devices=NCORE)
    xg_d = nc.declare_dram_parameter("xg", [128, XG_COLS], bf16, isOutput=False)
    wp_d = nc.declare_dram_parameter("wp", [128, WP_COLS], bf16, isOutput=False)
    out_d = nc.declare_dram_parameter("out", [128, NPHASE * NPAIR], f32,
                                      isOutput=True)

    with tile.TileContext(nc) as tc:
        with (
            tc.tile_pool(name="xg", bufs=1) as xgp,
            tc.tile_pool(name="wp", bufs=WP_BUFS) as wpp,
            tc.tile_pool(name="ps", bufs=4, space="PSUM") as psp,
            tc.tile_pool(name="outp", bufs=1) as outp,
        ):
            # the first two phase-0 xg pieces ride the fast HWDGE rings
            # ahead of the weight chunks so the first matmuls start ASAP;
            # the rest streams on the gpsimd (SWDGE) queue in the
            # background.
            xg_ts = []
            qp0 = QP[0]
            pieces = []
            for j in range(NJ):
                pj = xgp.tile([128, B * qp0], bf16, tag=f"xg0_{j}")
                eng = (nc.sync, nc.scalar, nc.gpsimd, nc.gpsimd,
                       nc.gpsimd)[j]
                eng.dma_start(
                    pj[:], xg_d[:, xoff[0] + j * B * qp0:
                                xoff[0] + (j + 1) * B * qp0])
                pieces.append(pj)
            xg_ts.append(pieces)

            # leading wp chunks have uniform boundaries across cores, so
            # their DMAs can issue before the switch dispatch
            wp_pre = []
            for k in range(WP_HOIST):
                t = wpp.tile([128, WP_CHUNK], bf16, tag="wp")
                eng = nc.sync if k % 2 == 0 else nc.scalar
                eng.dma_start(t[:, :ubounds[k + 1] - ubounds[k]],
                              wp_d[:, ubounds[k]:ubounds[k + 1]])
                wp_pre.append(t)

            for s in range(1, NPHASE):
                qp = QP[s]
                seg = xgp.tile([128, NJ * B * qp], bf16, tag=f"xg{s}")
                nc.gpsimd.dma_start(seg[:], xg_d[:, xoff[s]:xoff[s + 1]])
                xg_ts.append(seg)
            out_t = outp.tile([128, NPHASE * NPAIR], f32)

            ET = mybir.EngineType
            index = {
                ET.PE: nc.tensor.partition_id(),
                ET.SP: nc.sync.partition_id(),
                ET.Activation: nc.scalar.partition_id(),
                ET.DVE: nc.vector.partition_id(),
                ET.Pool: nc.gpsimd.partition_id(),
            }
            for core in tc.Switch(index, NCORE):
                descs = core_descs[core]
                wp_end = plan["core_wp_cols"][core]
                cidx = 0
                wp_t = None
                for s in range(len(descs)):
                    r = rows_of[core][s]
                    h = halos[r]
                    qp = QP[s]
                    acc = psp.tile([128, NPAIR], f32)
                    ph = descs[s]
                    for i, (kind, j, dp, col) in enumerate(ph):
                        ncols = 2 * O if kind == "P" else O
                        while col >= ubounds[cidx]:
                            if cidx < WP_HOIST:
                                wp_t = wp_pre[cidx]
                            else:
                                ccols = (min(ubounds[cidx + 1], wp_end)
                                         - ubounds[cidx])
                                wp_t = wpp.tile([128, WP_CHUNK], bf16,
                                                tag="wp")
                                eng = nc.sync if cidx % 2 == 0 else nc.scalar
                                eng.dma_start(
                                    wp_t[:, :ccols],
                                    wp_d[:, ubounds[cidx]:
                                         ubounds[cidx] + ccols])
                            cidx += 1
                        coff = col - ubounds[cidx - 1]
                        lhsT = wp_t[:, coff:coff + ncols]
                        if s == 0:
                            xv = xg_ts[0][j]
                            base = B * (h + dp)
                        else:
                            xv = xg_ts[s]
                            base = j * B * qp + B * (h + dp)
                        if kind == "P":
                            rhs = xv[:, base: base + NPAIR]
                            out_ap = acc[:, :]
                        else:
                            rhs = xv[:, base: base + B * P]
                            out_ap = acc[0:O, 0:B * P]
                        nc.tensor.matmul(out_ap, lhsT, rhs,
                                         start=(i == 0), stop=(i == len(ph) - 1))
                    nc.vector.tensor_copy(
                        out_t[:, s * NPAIR:(s + 1) * NPAIR], acc[:, :])
                    # late-phase outputs take the (by then idle) HWDGE
                    # rings so the tail isn't gated on the slow SWDGE queue
                    if s >= len(descs) - 2:
                        oeng = nc.scalar if s % 2 == 0 else nc.sync
                    else:
                        oeng = nc.gpsimd
                    oeng.dma_start(
                        out_d[:, s * NPAIR:(s + 1) * NPAIR],
                        out_t[:, s * NPAIR:(s + 1) * NPAIR])

    nc.compile()
    return nc


def _get_nc():
    global _NC
    if _NC is None:
        _NC = _build_nc(_get_plan())
    return _NC


def _build_core_inputs(plan, x, weight):
    import ml_dtypes

    psi = plan["psi"]
    ti_idx = plan["ti_idx"]
    rows_of = plan["rows_of"]
    row_groups = plan["row_groups"]
    halos = plan["halos"]
    QP = plan["QP"]
    xoff = plan["xoff"]
    XG_COLS = plan["xg_cols"]
    WP_COLS = plan["wp_cols"]
    core_descs = plan["core_descs"]

    wk = np.ascontiguousarray(weight.transpose(2, 1, 0)).reshape(K, C, O)

    xgs, wps = [], []
    for core in range(NCORE):
        descs = core_descs[core]
        # tap coefficient table + target column per 64-col block
        blk_cols = []
        coef_list = []  # [2, K] per 64-col block
        for s, ph in enumerate(descs):
            r = rows_of[core][s]
            groups = row_groups[r]
            for kind, j, dp, col in ph:
                dps = (dp, dp + 1) if kind == "P" else (dp,)
                for bi, dp_ in enumerate(dps):
                    p = dp_ % P
                    cf = np.zeros((2, K), dtype=np.float32)
                    for m in range(2):
                        w_ = groups[j][m] if m < len(groups[j]) else None
                        if w_ is not None:
                            cf[m] = psi[:, r, w_, p]
                    coef_list.append(cf)
                    blk_cols.append(col + bi * O)
        coef = np.stack(coef_list)  # [nblk, 2, K]
        blocks = np.einsum("tmk,kco->tmco", coef, wk,
                           optimize=True).reshape(-1, 128, O)
        wp_full = np.zeros((128, WP_COLS), dtype=np.float32)
        for bi, col in enumerate(blk_cols):
            wp_full[:, col:col + O] = blocks[bi]
        wps.append(wp_full.astype(ml_dtypes.bfloat16))

        xg = np.zeros((128, XG_COLS), dtype=np.float32)
        for s, ph in enumerate(descs):
            r = rows_of[core][s]
            groups = row_groups[r]
            h = halos[r]
            qp_r = P + 2 * h
            qq = (np.arange(qp_r) - h) % P
            for j, members in enumerate(groups):
                for m in range(2):
                    w_ = members[m] if m < len(members) else None
                    if w_ is None:
                        continue
                    ti = ti_idx[r, w_]
                    blk = x[:, :, ti, :][:, :, qq]  # [b, c, qp_r]
                    base = xoff[s] + j * B * QP[s]
                    xg[m * 64:(m + 1) * 64, base: base + B * qp_r] = (
                        blk.transpose(1, 2, 0).reshape(C, qp_r * B))
        xgs.append(xg.astype(ml_dtypes.bfloat16))
    return xgs, wps


def kernel(x, weight):
    from concourse.bass_utils import run_bass_kernel_spmd

    x = np.ascontiguousarray(np.asarray(x, dtype=np.float32))
    weight = np.ascontiguousarray(np.asarray(weight, dtype=np.float32))
    plan = _get_plan()
    nc = _get_nc()
    xgs, wps = _build_core_inputs(plan, x, weight)
    in_maps = [{"xg": xgs[i], "wp": wps[i]} for i in range(NCORE)]
    res = run_bass_kernel_spmd(nc, in_maps, list(range(NCORE)))

    out = np.zeros((B, O, NLAT, P), dtype=np.float32)
    rows_of = plan["rows_of"]
    for core in range(NCORE):
        oc = np.asarray(res.results[core]["out"]).reshape(128, NPHASE, NPAIR)
        merged = oc[0:O, :, 0:B * P] + oc[O:128, :, B:B * P + B]
        mg = merged.reshape(O, NPHASE, P, B)
        for s, r in enumerate(rows_of[core]):
            out[:, :, r, :] = mg[:, s, :, :].transpose(2, 0, 1)
    return out


def _numpy_sim(x, weight):
    """Host replica of the device program (for validation)."""
    plan = _get_plan()
    xgs, wps = _build_core_inputs(plan, x, weight)
    rows_of = plan["rows_of"]
    halos = plan["halos"]
    QP = plan["QP"]
    xoff = plan["xoff"]
    core_descs = plan["core_descs"]
    out = np.zeros((B, O, NLAT, P), dtype=np.float32)
    for core in range(NCORE):
        xg = xgs[core].astype(np.float32)
        wp = wps[core].astype(np.float32)
        descs = core_descs[core]
        for s, ph in enumerate(descs):
            r = rows_of[core][s]
            h = halos[r]
            qp = QP[s]
            acc = np.zeros((128, NPAIR), dtype=np.float32)
            for (kind, j, dp, col) in ph:
                base = xoff[s] + j * B * qp + B * (h + dp)
                if kind == "P":
                    acc += wp[:, col:col + 2 * O].T @ xg[:, base: base + NPAIR]
                else:
                    acc[0:O, 0:B * P] += (
                        wp[:, col:col + O].T @ xg[:, base: base + B * P])
            merged = acc[0:O, 0:B * P] + acc[O:128, B:B * P + B]
            out[:, :, r, :] = merged.reshape(O, P, B).transpose(2, 0, 1)
    return out


if __name__ == "__main__":
    plan = _get_plan()
    for c in range(NCORE):
        ph = plan["core_descs"][c]
        npair_ = sum(1 for p_ in ph for d in p_ if d[0] == "P")
        nsgl = sum(1 for p_ in ph for d in p_ if d[0] == "S")
        cyc = npair_ * NPAIR + nsgl * B * P
        print(f"core {c}: rows={len(ph)} pairs={npair_} singles={nsgl} "
              f"cyc={cyc} -> {cyc*0.41667/1e3:.1f}us wp_MB="
              f"{plan['core_wp_cols'][c]*128*2/1e6:.1f}")
    print("xg MB:", plan["xg_cols"] * 128 * 2 / 1e6,
          "wp decl MB:", plan["wp_cols"] * 128 * 2 / 1e6)
    d = np.load("/tmp/ref_io.npz")
    got = _numpy_sim(d["x"], d["weight"])
    exp = d["expected"]
    rel = np.linalg.norm((got - exp).ravel()) / np.linalg.norm(exp.ravel())
    print("numpy-sim rel err:", rel)
